# revision 25
# baseline (speedup 1.0000x reference)
"""Any4 quantized linear (LUT dequant + GEMM + bias) on 8 Trainium2 cores.

Strategy: column-parallel over out_features O=4096 -> OSH=512 per core.

Host prep (layout only, no reference FLOPs): x is transposed/cast once to
xT bf16 [I, M] so the device streams contraction-major tiles with no
on-chip transposes; codes are sent as an exact bf16 plane plus three uint8
bit-planes (bits 1-3); the 16-entry LUT is repacked into 8 interpolation
pairs (delta_j, gamma_j) so bit0 resolves arithmetically:
  u_j = c*delta_j + gamma_j  ->  u_j == lut[c] whenever c>>1 == j.
The output is produced transposed ([OSH, M] bf16) and re-assembled on host.

Device per core:
  dequant prologue, pipelined in 8 (k-half, o-tile) units of [128, 2048]
  (unit 0 split 256/256/512/1024 to cut lead-in latency):
    ACT:  7 of 8 u_j planes (Identity, per-partition scale/bias)
    DVE:  1 u_j plane + 7 in-place copy_predicated tree merges (the
          hard floor: copy_predicated runs at 1 elem/cycle, no 2x mode)
          + 2 of 16 per-group scale/zero affines
    Pool: 14 of 16 per-group affines (small tensor_scalar)
    sync: codes/pred/table loads, chunked xbar DMA-transposes into
          resident wt tiles, and block-0 x phases, interleaved so waits
          never stall an engine with pending compute
  main loop (8 m-blocks of 1024):
    x k-tiles [128, 1024] bf16 streamed from HBM on the sync HWDGE queue;
    per k: 4 stationary weight loads (one per o-subtile) x 2 moving
    matmuls of 512 columns accumulating into 8 PSUM banks (PSUM caps
    in-flight rows at 1024, so exactly one block overlaps the prologue);
    PSUM->SBUF copy on ACT fuses the per-partition bias and casts to
    bf16; y stored as [OSH, M]. Block 0 runs o-subtile-outer in two
    k-phases so the PE chases the dequant pipeline unit by unit, and its
    copies are emitted inline so PSUM banks free the moment block 1 starts.

Self-contained: hardcodes shapes M=8192, I=4096, O=4096, G=128, n_cores=8.
"""
import sys

sys.path.insert(0, "/opt/trn_rl_repo")

import numpy as np
import ml_dtypes

import concourse.bass as bass
import concourse.mybir as mybir
import bass_rust
from concourse import tile
from concourse.bass_utils import run_bass_kernel_spmd

M, I, O, G = 8192, 4096, 4096, 128
NCORES = 8
OSH = O // NCORES          # 512 out features per core
P = 128                    # partitions
KT = I // P                # 32 k-tiles
OT = OSH // P              # 4 o-subtiles
NG = I // G                # 32 scale groups
MBLK = 1024                # m-block columns in the main loop
NMB = M // MBLK            # 8 m-blocks
CH = 1024                  # dequant column chunk
NCH = I // CH              # 4 chunks per o-tile
CG = CH // P               # 8 k-tiles per chunk
BF = mybir.dt.bfloat16
F32 = mybir.dt.float32
U8 = mybir.dt.uint8
U16 = mybir.dt.uint16
U32 = mybir.dt.uint32
AF = mybir.ActivationFunctionType
OP = mybir.AluOpType


def _split_waits(nc, budget=1, noop_budget=1):
    """walrus in this toolchain rejects instructions with >1 embedded sem
    wait; move excess waits onto same-engine NoOp carriers placed directly
    before the instruction."""
    ctr = 0
    for fn in nc.m.functions:
        for bb in fn.blocks:
            lst = bb.instructions
            i = 0
            while i < len(lst):
                inst = lst[i]
                si = inst.sync_info
                if si is None:
                    i += 1
                    continue
                waits = list(si.on_wait or [])
                if len(waits) <= budget:
                    i += 1
                    continue
                inst.sync_info = bass_rust.SyncInfo(
                    on_wait=waits[:budget], on_update=list(si.on_update or []))
                excess = waits[budget:]
                cars = []
                for j in range(0, len(excess), noop_budget):
                    ctr += 1
                    n = mybir.InstNoOp(name=f"waitc-{ctr}", ins=[], outs=[])
                    n.engine = inst.engine
                    n.sync_info = bass_rust.SyncInfo(
                        on_wait=excess[j:j + noop_budget], on_update=[])
                    cars.append(n)
                for j, c in enumerate(cars):
                    lst.insert(i + j, c)
                i += 1 + len(cars)
    return ctr


def build(nmb=NMB):
    nc = bass.Bass()
    xT_d = nc.dram_tensor("xT", [I, M], BF, kind="ExternalInput")
    cpl_d = nc.dram_tensor("cpl", [OSH, I], BF, kind="ExternalInput")
    predq_d = nc.dram_tensor("predq", [OSH, 3, I], U8, kind="ExternalInput")
    tbl_d = nc.dram_tensor("tbl", [OSH, 16 + 2 * NG], F32,
                           kind="ExternalInput")
    bias_d = nc.dram_tensor("bias", [P, OT], F32, kind="ExternalInput")
    yt_d = nc.dram_tensor("yt", [OSH, M], BF, kind="ExternalOutput")

    with tile.TileContext(nc) as tc:
        with (
            tc.tile_pool(name="const", bufs=1) as cpool,
            tc.tile_pool(name="tbl", bufs=1) as tbl,
            tc.tile_pool(name="cplp", bufs=3) as cplp,
            tc.tile_pool(name="dq", bufs=2) as dq,
            tc.tile_pool(name="xp", bufs=12) as xp,
            tc.tile_pool(name="yp", bufs=2) as ypool,
            tc.tile_pool(name="psy", bufs=1, space="PSUM") as psy,
        ):
            UW = I // 2   # dequant unit width (one k-half of an o-tile)
            UK = UW // P  # 16 k-tiles per unit

            # resident transposed weights, one tile per (o-tile, k-half):
            # wt_th[t][h][p, kc*128 + q] = W[t*128 + q, (h*16+kc)*128 + p]
            wt_th = [[cpool.tile([P, UW], BF, tag=f"wt{t}_{h}",
                                 name=f"wt{t}_{h}")
                      for h in range(2)] for t in range(OT)]

            # ---------------- dequant (8 units, k-half-major) -------------
            # pair-tree: u_j = c*delta_j + gamma_j (j = c>>1), then 7
            # in-place copy_predicated merges on bit1/bit2/bit3.
            units = [(h, t) for h in range(2) for t in range(OT)]
            # unit 0 is split into two half-width sub-units to cut the
            # pipeline lead-in latency before the first matmul.
            subunits = []
            for n, (h, t) in enumerate(units):
                if n == 0:
                    subunits += [(h, t, 0, 256), (h, t, 256, 256),
                                 (h, t, 512, 512), (h, t, 1024, 1024)]
                else:
                    subunits.append((h, t, 0, UW))

            def emit_su_loads(m):
                h, t, off, w = subunits[m]
                osl = slice(t * P, (t + 1) * P)
                hs = slice(h * UW + off, h * UW + off + w)
                cq = cplp.tile([P, UW], BF, tag="cq", name="cq")
                nc.sync.dma_start(cq[:, 0:w], cpl_d[osl, hs])
                pq = cplp.tile([P, 3, UW], U8, tag="pq", name="pq")
                nc.sync.dma_start(pq[:, :, 0:w], predq_d[osl, :, hs])
                return cq, pq

            loads = [emit_su_loads(0)]
            bias_sb = cpool.tile([P, OT], F32, tag="bias", name="bias")
            nc.sync.dma_start(bias_sb[:], bias_d[:])
            tbs = []
            for t in range(OT):
                tb = tbl.tile([P, 16 + 2 * NG], F32, tag=f"tb{t}",
                              name=f"tb{t}")
                nc.sync.dma_start(tb[:], tbl_d[t * P:(t + 1) * P, :])
                tbs.append(tb)
            loads.append(emit_su_loads(1))

            def mm_step(k, s, yph, xk, start, stop):
                h = k // UK
                lo = (k % UK) * P
                for half in range(2):
                    nc.tensor.matmul(
                        yph[half][:], wt_th[s][h][:, lo:lo + P],
                        xk[:, half * 512:(half + 1) * 512],
                        start=start, stop=stop)

            yps0 = [[psy.tile([P, 512], F32, tag=f"y{s}_{hh}",
                              name=f"y{s}_{hh}")
                     for hh in range(2)] for s in range(OT)]

            # prefetch block-0's first x phase so the PE can start at the
            # first transpose (xp bufs must cover these plus lookahead)
            NPRE = 10
            xk_pre = []
            for k in range(NPRE):
                xk = xp.tile([P, MBLK], BF, tag="xk", name="xk")
                nc.sync.dma_start(xk[:], xT_d[k * P:(k + 1) * P, 0:MBLK])
                xk_pre.append(xk)

            for m, (h, t, off, w) in enumerate(subunits):
                cq, pq = loads[m]
                tb = tbs[t]
                cqv = cq[:, 0:w]
                # packed u-planes: pk[a] lanes = (lo: u_a, hi: u_{a+4}),
                # so each merge level's single predicate drives a 4-byte
                # copy_predicated moving two bf16 winners per element-cycle.
                pk = [dq.tile([P, UW, 2], BF, tag=f"pk{a}", name=f"pk{a}")
                      for a in range(4)]
                for j in range(8):
                    a, lane = j % 4, j // 4
                    dst = pk[a][:, 0:w, lane]
                    if j < 1:
                        nc.vector.tensor_scalar(
                            dst, cqv, tb[:, j:j + 1],
                            tb[:, 8 + j:9 + j], OP.mult, OP.add)
                    else:
                        nc.scalar.activation(
                            dst, cqv, AF.Identity,
                            bias=tb[:, 8 + j:9 + j], scale=tb[:, j:j + 1])

                def v32(a):
                    return pk[a][:, 0:w, :].rearrange(
                        "p w l -> p (w l)").bitcast(U32)

                nc.vector.copy_predicated(v32(0), pq[:, 0, 0:w], v32(1))
                nc.vector.copy_predicated(v32(2), pq[:, 0, 0:w], v32(3))
                nc.vector.copy_predicated(v32(0), pq[:, 1, 0:w], v32(2))
                nc.vector.copy_predicated(pk[0][:, 0:w, 0], pq[:, 2, 0:w],
                                          pk[0][:, 0:w, 1])

                # per-group affine: w = wlut * scale[g] + zero[g]
                # (12 groups on Pool, 4 on ACT per full unit)
                wb = dq.tile([P, UW], BF, tag="wb", name="wb")
                for gi in range(w // G):
                    g = (h * UW + off) // G + gi
                    gs = slice(gi * G, (gi + 1) * G)
                    eng = nc.vector if gi % 8 == 7 else nc.gpsimd
                    eng.tensor_scalar(
                        wb[:, gs], pk[0][:, gs, 0], tb[:, 16 + g:17 + g],
                        tb[:, 16 + NG + g:17 + NG + g], OP.mult, OP.add)

                # sync segment: prefetch sub-unit m+2 loads, transpose,
                # then block-0's x phase
                if m + 2 < len(subunits):
                    loads.append(emit_su_loads(m + 2))
                tstep = min(w, CH)
                for so in range(0, w, tstep):
                    nc.sync.dma_start_transpose(
                        wt_th[t][h][:, off + so:off + so + tstep].rearrange(
                            "p (k q) -> p k q", k=tstep // P),
                        wb[:, so:so + tstep])
                s = t
                k0 = (h * UW + off) // P
                for k in range(k0, k0 + w // P):
                    if k < NPRE:
                        xk = xk_pre[k]
                    else:
                        xk = xp.tile([P, MBLK], BF, tag="xk", name="xk")
                        nc.sync.dma_start(
                            xk[:], xT_d[k * P:(k + 1) * P, 0:MBLK])
                    mm_step(k, s, yps0[s], xk,
                            start=(k == 0), stop=(k == KT - 1))
                if h == 1:
                    # osub s of block 0 is complete: copy+store now so its
                    # PSUM banks are free the moment block 1 starts
                    ysb = ypool.tile([P, MBLK], BF, tag="ysb", name="ysb")
                    for hh in range(2):
                        nc.scalar.activation(
                            ysb[:, hh * 512:(hh + 1) * 512], yps0[s][hh][:],
                            AF.Identity, bias=bias_sb[:, s:s + 1], scale=1.0)
                    nc.scalar.dma_start(
                        yt_d[s * P:(s + 1) * P, 0:MBLK], ysb[:])

            # ---------------- main loop (blocks 1..7) ----------------
            for b in range(1, nmb):
                mo = b * MBLK
                yps = [[psy.tile([P, 512], F32, tag=f"y{s}_{hh}",
                                 name=f"y{s}_{hh}")
                        for hh in range(2)] for s in range(OT)]
                for k in range(KT):
                    xk = xp.tile([P, MBLK], BF, tag="xk", name="xk")
                    nc.sync.dma_start(
                        xk[:], xT_d[k * P:(k + 1) * P, mo:mo + MBLK])
                    for s in range(OT):
                        mm_step(k, s, yps[s], xk,
                                start=(k == 0), stop=(k == KT - 1))
                for s in range(OT):
                    ysb = ypool.tile([P, MBLK], BF, tag="ysb", name="ysb")
                    for hh in range(2):
                        nc.scalar.activation(
                            ysb[:, hh * 512:(hh + 1) * 512], yps[s][hh][:],
                            AF.Identity, bias=bias_sb[:, s:s + 1], scale=1.0)
                    nc.scalar.dma_start(
                        yt_d[s * P:(s + 1) * P, mo:mo + MBLK], ysb[:])

    _split_waits(nc)
    return nc


def _install_ntff_shim():
    """This image's antenv lacks axon_hooks, so run_bass_kernel_spmd's
    trace=True path can't find the NTFF profile hook. Recreate it: a tiny
    antenv.axon_hooks module plus the ctypes hook into libaxon_pjrt.so
    (same mechanism as trn_agent_boot)."""
    import types, contextlib, ctypes, os as _os
    if "antenv.axon_hooks" in sys.modules:
        return
    mod = types.ModuleType("antenv.axon_hooks")
    holder = {}
    mod.set_axon_ntff_profile_hook = lambda h: holder.__setitem__("h", h)
    mod.get_axon_ntff_profile_hook = lambda: holder.get("h")
    sys.modules["antenv.axon_hooks"] = mod
    try:
        import antenv
        antenv.axon_hooks = mod
    except ImportError:
        pass
    so_path = "/opt/axon/libaxon_pjrt.so"
    if not _os.path.exists(so_path):
        return
    lib = ctypes.CDLL(so_path)
    if not hasattr(lib, "axon_start_nrt_profile"):
        return
    lib.axon_start_nrt_profile.argtypes = [
        ctypes.POINTER(ctypes.c_int64), ctypes.c_size_t]
    lib.axon_start_nrt_profile.restype = ctypes.c_int64
    lib.axon_stop_nrt_profile.argtypes = [ctypes.c_char_p]
    lib.axon_stop_nrt_profile.restype = ctypes.c_int64

    @contextlib.contextmanager
    def _hook(output_dir, device_ids):
        import jax
        jax.devices()
        if device_ids:
            ids = (ctypes.c_int64 * len(device_ids))(*device_ids)
            rc = lib.axon_start_nrt_profile(ids, len(device_ids))
        else:
            rc = lib.axon_start_nrt_profile(None, 0)
        if rc != 0:
            raise RuntimeError(f"axon_start_nrt_profile rc={rc}")
        try:
            yield
        finally:
            n = lib.axon_stop_nrt_profile(str(output_dir).encode())
            print(f"ntff profile: {n} file(s) written to {output_dir}")

    mod.set_axon_ntff_profile_hook(_hook)


_NC_CACHE = None
_BUILD_KW = {}


def _get_nc():
    global _NC_CACHE
    if _NC_CACHE is None:
        _NC_CACHE = build(**_BUILD_KW)
    return _NC_CACHE


def _make_in_maps(input, weight, lut, scales_and_zeros, bias):
    bf16 = ml_dtypes.bfloat16
    x = np.asarray(input, dtype=np.float32)
    xT = np.ascontiguousarray(x.T).astype(bf16)  # [I, M]
    codes = np.asarray(weight, dtype=np.int32)
    lut = np.asarray(lut, dtype=np.float32)
    sz = np.asarray(scales_and_zeros, dtype=np.float32)
    bias = np.asarray(bias, dtype=np.float32)
    scaleT = np.ascontiguousarray(sz[..., 0].T)  # [O, I//G]
    zeroT = np.ascontiguousarray(sz[..., 1].T)

    # pair interpolation tables: u_j = c*delta_j + gamma_j (j = c>>1)
    base = lut[:, 0::2]                          # [O, 8]
    dtab = lut[:, 1::2] - base                   # delta_j
    gtab = base - dtab * (2.0 * np.arange(8, dtype=np.float32))

    in_maps = []
    for c in range(NCORES):
        osl = slice(c * OSH, (c + 1) * OSH)
        cs = codes[osl]
        predq = np.empty((OSH, 3, I), dtype=np.uint8)
        predq[:, 0, :] = (cs >> 1) & 1                    # bit1
        predq[:, 1, :] = (cs >> 2) & 1                    # bit2
        predq[:, 2, :] = (cs >> 3) & 1                    # bit3
        tblv = np.concatenate([dtab[osl], gtab[osl],
                               scaleT[osl], zeroT[osl]],
                              axis=1).astype(np.float32)
        in_maps.append({
            "xT": xT,
            "cpl": np.ascontiguousarray(cs).astype(bf16),
            "predq": predq,
            "tbl": np.ascontiguousarray(tblv),
            "bias": np.ascontiguousarray(
                bias[osl].reshape(OT, P).T),
        })
    return in_maps


def run(input, weight, lut, scales_and_zeros, bias, trace=False, tmpdir=None):
    if trace:
        _install_ntff_shim()
        import concourse.bass_utils as _bu
        _bu.upload_artifacts = lambda d: d  # zero-egress container
    nc = _get_nc()
    in_maps = _make_in_maps(input, weight, lut, scales_and_zeros, bias)
    res = run_bass_kernel_spmd(
        nc, in_maps, list(range(NCORES)), trace=trace, tmpdir=tmpdir)
    y = np.empty((M, O), dtype=np.float32)
    for c in range(NCORES):
        yt = np.asarray(res.results[c]["yt"])  # [OSH, M] bf16
        y[:, c * OSH:(c + 1) * OSH] = yt.astype(np.float32).T
    return y, res


def kernel(input, weight, lut, scales_and_zeros, bias):
    orig_shape = np.asarray(input).shape
    y, _ = run(input, weight, lut, scales_and_zeros, bias, trace=False)
    return y.reshape(*orig_shape[:-1], O)


# revision 26
# speedup vs baseline: 1.0463x; 1.0463x over previous
"""Any4 quantized linear (LUT dequant + GEMM + bias) on 8 Trainium2 cores.

Strategy: column-parallel over out_features O=4096 -> OSH=512 per core.

Host prep (layout only, no reference FLOPs): x is transposed/cast once to
xT bf16 [I, M] so the device streams contraction-major tiles with no
on-chip transposes; codes are sent as an exact bf16 plane plus three uint8
bit-planes (bits 1-3); the 16-entry LUT is repacked into 8 interpolation
pairs (delta_j, gamma_j) so bit0 resolves arithmetically:
  u_j = c*delta_j + gamma_j  ->  u_j == lut[c] whenever c>>1 == j.
The output is produced transposed ([OSH, M] bf16) and re-assembled on host.

Device per core:
  dequant prologue, pipelined in 8 (k-half, o-tile) units of [128, 2048]
  (unit 0 split 256/256/512/1024 to cut lead-in latency):
    ACT:  7 of 8 u_j planes (Identity, per-partition scale/bias)
    DVE:  1 u_j plane + 7 in-place copy_predicated tree merges (the
          hard floor: copy_predicated runs at 1 elem/cycle, no 2x mode)
          + 2 of 16 per-group scale/zero affines
    Pool: 14 of 16 per-group affines (small tensor_scalar)
    sync: codes/pred/table loads, chunked xbar DMA-transposes into
          resident wt tiles, and block-0 x phases, interleaved so waits
          never stall an engine with pending compute
  main loop (8 m-blocks of 1024):
    x k-tiles [128, 1024] bf16 streamed from HBM on the sync HWDGE queue;
    per k: 4 stationary weight loads (one per o-subtile) x 2 moving
    matmuls of 512 columns accumulating into 8 PSUM banks (PSUM caps
    in-flight rows at 1024, so exactly one block overlaps the prologue);
    PSUM->SBUF copy on ACT fuses the per-partition bias and casts to
    bf16; y stored as [OSH, M]. Block 0 runs o-subtile-outer in two
    k-phases so the PE chases the dequant pipeline unit by unit, and its
    copies are emitted inline so PSUM banks free the moment block 1 starts.

Self-contained: hardcodes shapes M=8192, I=4096, O=4096, G=128, n_cores=8.
"""
import sys

sys.path.insert(0, "/opt/trn_rl_repo")

import numpy as np
import ml_dtypes

import concourse.bass as bass
import concourse.mybir as mybir
import bass_rust
from concourse import tile
from concourse.bass_utils import run_bass_kernel_spmd

M, I, O, G = 8192, 4096, 4096, 128
NCORES = 8
OSH = O // NCORES          # 512 out features per core
P = 128                    # partitions
KT = I // P                # 32 k-tiles
OT = OSH // P              # 4 o-subtiles
NG = I // G                # 32 scale groups
MBLK = 1024                # m-block columns in the main loop
NMB = M // MBLK            # 8 m-blocks
CH = 1024                  # dequant column chunk
NCH = I // CH              # 4 chunks per o-tile
CG = CH // P               # 8 k-tiles per chunk
BF = mybir.dt.bfloat16
F32 = mybir.dt.float32
U8 = mybir.dt.uint8
U16 = mybir.dt.uint16
U32 = mybir.dt.uint32
AF = mybir.ActivationFunctionType
OP = mybir.AluOpType


def _split_waits(nc, budget=1, noop_budget=1):
    """walrus in this toolchain rejects instructions with >1 embedded sem
    wait; move excess waits onto same-engine NoOp carriers placed directly
    before the instruction."""
    ctr = 0
    for fn in nc.m.functions:
        for bb in fn.blocks:
            lst = bb.instructions
            i = 0
            while i < len(lst):
                inst = lst[i]
                si = inst.sync_info
                if si is None:
                    i += 1
                    continue
                waits = list(si.on_wait or [])
                if len(waits) <= budget:
                    i += 1
                    continue
                inst.sync_info = bass_rust.SyncInfo(
                    on_wait=waits[:budget], on_update=list(si.on_update or []))
                excess = waits[budget:]
                cars = []
                for j in range(0, len(excess), noop_budget):
                    ctr += 1
                    n = mybir.InstNoOp(name=f"waitc-{ctr}", ins=[], outs=[])
                    n.engine = inst.engine
                    n.sync_info = bass_rust.SyncInfo(
                        on_wait=excess[j:j + noop_budget], on_update=[])
                    cars.append(n)
                for j, c in enumerate(cars):
                    lst.insert(i + j, c)
                i += 1 + len(cars)
    return ctr


def build(nmb=NMB):
    nc = bass.Bass()
    xT_d = nc.dram_tensor("xT", [I, M], BF, kind="ExternalInput")
    cpl_d = nc.dram_tensor("cpl", [OSH, I], BF, kind="ExternalInput")
    predq_d = nc.dram_tensor("predq", [OSH, 3, I], U8, kind="ExternalInput")
    tbl_d = nc.dram_tensor("tbl", [OSH, 16 + 2 * NG], F32,
                           kind="ExternalInput")
    bias_d = nc.dram_tensor("bias", [P, OT], F32, kind="ExternalInput")
    yt_d = nc.dram_tensor("yt", [OSH, M], BF, kind="ExternalOutput")

    with tile.TileContext(nc) as tc:
        with (
            tc.tile_pool(name="const", bufs=1) as cpool,
            tc.tile_pool(name="tbl", bufs=1) as tbl,
            tc.tile_pool(name="cplp", bufs=3) as cplp,
            tc.tile_pool(name="dq", bufs=2) as dq,
            tc.tile_pool(name="xp", bufs=12) as xp,
            tc.tile_pool(name="yp", bufs=2) as ypool,
            tc.tile_pool(name="psy", bufs=1, space="PSUM") as psy,
        ):
            UW = I // 2   # dequant unit width (one k-half of an o-tile)
            UK = UW // P  # 16 k-tiles per unit

            # resident transposed weights, one tile per (o-tile, k-half):
            # wt_th[t][h][p, kc*128 + q] = W[t*128 + q, (h*16+kc)*128 + p]
            wt_th = [[cpool.tile([P, UW], BF, tag=f"wt{t}_{h}",
                                 name=f"wt{t}_{h}")
                      for h in range(2)] for t in range(OT)]

            # ---------------- dequant (8 units, k-half-major) -------------
            # pair-tree: u_j = c*delta_j + gamma_j (j = c>>1), then 7
            # in-place copy_predicated merges on bit1/bit2/bit3.
            units = [(h, t) for h in range(2) for t in range(OT)]
            # unit 0 is split into two half-width sub-units to cut the
            # pipeline lead-in latency before the first matmul.
            subunits = []
            for n, (h, t) in enumerate(units):
                if n == 0:
                    subunits += [(h, t, 0, 256), (h, t, 256, 256),
                                 (h, t, 512, 512), (h, t, 1024, 1024)]
                else:
                    subunits.append((h, t, 0, UW))

            def emit_su_loads(m):
                h, t, off, w = subunits[m]
                osl = slice(t * P, (t + 1) * P)
                hs = slice(h * UW + off, h * UW + off + w)
                cq = cplp.tile([P, UW], BF, tag="cq", name="cq")
                nc.sync.dma_start(cq[:, 0:w], cpl_d[osl, hs])
                pq = cplp.tile([P, 3, UW], U8, tag="pq", name="pq")
                nc.sync.dma_start(pq[:, :, 0:w], predq_d[osl, :, hs])
                return cq, pq

            loads = [emit_su_loads(0)]
            bias_sb = cpool.tile([P, OT], F32, tag="bias", name="bias")
            nc.sync.dma_start(bias_sb[:], bias_d[:])
            tbs = []
            for t in range(OT):
                tb = tbl.tile([P, 16 + 2 * NG], F32, tag=f"tb{t}",
                              name=f"tb{t}")
                nc.sync.dma_start(tb[:], tbl_d[t * P:(t + 1) * P, :])
                tbs.append(tb)
            loads.append(emit_su_loads(1))

            def mm_step(k, s, yph, xk, start, stop):
                h = k // UK
                lo = (k % UK) * P
                for half in range(2):
                    nc.tensor.matmul(
                        yph[half][:], wt_th[s][h][:, lo:lo + P],
                        xk[:, half * 512:(half + 1) * 512],
                        start=start, stop=stop)

            yps0 = [[psy.tile([P, 512], F32, tag=f"y{s}_{hh}",
                              name=f"y{s}_{hh}")
                     for hh in range(2)] for s in range(OT)]

            # prefetch block-0's first x phase so the PE can start at the
            # first transpose (xp bufs must cover these plus lookahead)
            NPRE = 10
            xk_pre = []
            for k in range(NPRE):
                xk = xp.tile([P, MBLK], BF, tag="xk", name="xk")
                nc.sync.dma_start(xk[:], xT_d[k * P:(k + 1) * P, 0:MBLK])
                xk_pre.append(xk)

            for m, (h, t, off, w) in enumerate(subunits):
                cq, pq = loads[m]
                tb = tbs[t]
                cqv = cq[:, 0:w]
                # packed u-planes: pk[a] lanes = (lo: u_a, hi: u_{a+4}),
                # so each merge level's single predicate drives a 4-byte
                # copy_predicated moving two bf16 winners per element-cycle.
                pk = [dq.tile([P, UW, 2], BF, tag=f"pk{a}", name=f"pk{a}")
                      for a in range(4)]
                for j in range(8):
                    a, lane = j % 4, j // 4
                    dst = pk[a][:, 0:w, lane]
                    if j < 2:
                        nc.vector.tensor_scalar(
                            dst, cqv, tb[:, j:j + 1],
                            tb[:, 8 + j:9 + j], OP.mult, OP.add)
                    else:
                        nc.scalar.activation(
                            dst, cqv, AF.Identity,
                            bias=tb[:, 8 + j:9 + j], scale=tb[:, j:j + 1])

                def v32(a):
                    return pk[a][:, 0:w, :].rearrange(
                        "p w l -> p (w l)").bitcast(U32)

                nc.vector.copy_predicated(v32(0), pq[:, 0, 0:w], v32(1))
                nc.vector.copy_predicated(v32(2), pq[:, 0, 0:w], v32(3))
                nc.vector.copy_predicated(v32(0), pq[:, 1, 0:w], v32(2))
                nc.vector.copy_predicated(pk[0][:, 0:w, 0], pq[:, 2, 0:w],
                                          pk[0][:, 0:w, 1])

                # per-group affine: w = wlut * scale[g] + zero[g]
                # (12 groups on Pool, 4 on ACT per full unit)
                wb = dq.tile([P, UW], BF, tag="wb", name="wb")
                for gi in range(w // G):
                    g = (h * UW + off) // G + gi
                    gs = slice(gi * G, (gi + 1) * G)
                    if gi in (4, 9, 14):
                        nc.vector.tensor_scalar(
                            wb[:, gs], pk[0][:, gs, 0], tb[:, 16 + g:17 + g],
                            tb[:, 16 + NG + g:17 + NG + g], OP.mult, OP.add)
                    elif gi in (2, 12):
                        nc.scalar.activation(
                            wb[:, gs], pk[0][:, gs, 0], AF.Identity,
                            bias=tb[:, 16 + NG + g:17 + NG + g],
                            scale=tb[:, 16 + g:17 + g])
                    else:
                        nc.gpsimd.tensor_scalar(
                            wb[:, gs], pk[0][:, gs, 0], tb[:, 16 + g:17 + g],
                            tb[:, 16 + NG + g:17 + NG + g], OP.mult, OP.add)

                # sync segment: prefetch sub-unit m+2 loads, transpose,
                # then block-0's x phase
                if m + 2 < len(subunits):
                    loads.append(emit_su_loads(m + 2))
                tstep = min(w, CH)
                for so in range(0, w, tstep):
                    nc.sync.dma_start_transpose(
                        wt_th[t][h][:, off + so:off + so + tstep].rearrange(
                            "p (k q) -> p k q", k=tstep // P),
                        wb[:, so:so + tstep])
                s = t
                k0 = (h * UW + off) // P
                for k in range(k0, k0 + w // P):
                    if k < NPRE:
                        xk = xk_pre[k]
                    else:
                        xk = xp.tile([P, MBLK], BF, tag="xk", name="xk")
                        nc.sync.dma_start(
                            xk[:], xT_d[k * P:(k + 1) * P, 0:MBLK])
                    mm_step(k, s, yps0[s], xk,
                            start=(k == 0), stop=(k == KT - 1))
                if h == 1:
                    # osub s of block 0 is complete: copy+store now so its
                    # PSUM banks are free the moment block 1 starts
                    ysb = ypool.tile([P, MBLK], BF, tag="ysb", name="ysb")
                    for hh in range(2):
                        nc.scalar.activation(
                            ysb[:, hh * 512:(hh + 1) * 512], yps0[s][hh][:],
                            AF.Identity, bias=bias_sb[:, s:s + 1], scale=1.0)
                    nc.scalar.dma_start(
                        yt_d[s * P:(s + 1) * P, 0:MBLK], ysb[:])

            # ---------------- main loop (blocks 1..7) ----------------
            for b in range(1, nmb):
                mo = b * MBLK
                yps = [[psy.tile([P, 512], F32, tag=f"y{s}_{hh}",
                                 name=f"y{s}_{hh}")
                        for hh in range(2)] for s in range(OT)]
                for k in range(KT):
                    xk = xp.tile([P, MBLK], BF, tag="xk", name="xk")
                    nc.sync.dma_start(
                        xk[:], xT_d[k * P:(k + 1) * P, mo:mo + MBLK])
                    for s in range(OT):
                        mm_step(k, s, yps[s], xk,
                                start=(k == 0), stop=(k == KT - 1))
                for s in range(OT):
                    ysb = ypool.tile([P, MBLK], BF, tag="ysb", name="ysb")
                    for hh in range(2):
                        nc.scalar.activation(
                            ysb[:, hh * 512:(hh + 1) * 512], yps[s][hh][:],
                            AF.Identity, bias=bias_sb[:, s:s + 1], scale=1.0)
                    nc.scalar.dma_start(
                        yt_d[s * P:(s + 1) * P, mo:mo + MBLK], ysb[:])

    _split_waits(nc)
    return nc


def _install_ntff_shim():
    """This image's antenv lacks axon_hooks, so run_bass_kernel_spmd's
    trace=True path can't find the NTFF profile hook. Recreate it: a tiny
    antenv.axon_hooks module plus the ctypes hook into libaxon_pjrt.so
    (same mechanism as trn_agent_boot)."""
    import types, contextlib, ctypes, os as _os
    if "antenv.axon_hooks" in sys.modules:
        return
    mod = types.ModuleType("antenv.axon_hooks")
    holder = {}
    mod.set_axon_ntff_profile_hook = lambda h: holder.__setitem__("h", h)
    mod.get_axon_ntff_profile_hook = lambda: holder.get("h")
    sys.modules["antenv.axon_hooks"] = mod
    try:
        import antenv
        antenv.axon_hooks = mod
    except ImportError:
        pass
    so_path = "/opt/axon/libaxon_pjrt.so"
    if not _os.path.exists(so_path):
        return
    lib = ctypes.CDLL(so_path)
    if not hasattr(lib, "axon_start_nrt_profile"):
        return
    lib.axon_start_nrt_profile.argtypes = [
        ctypes.POINTER(ctypes.c_int64), ctypes.c_size_t]
    lib.axon_start_nrt_profile.restype = ctypes.c_int64
    lib.axon_stop_nrt_profile.argtypes = [ctypes.c_char_p]
    lib.axon_stop_nrt_profile.restype = ctypes.c_int64

    @contextlib.contextmanager
    def _hook(output_dir, device_ids):
        import jax
        jax.devices()
        if device_ids:
            ids = (ctypes.c_int64 * len(device_ids))(*device_ids)
            rc = lib.axon_start_nrt_profile(ids, len(device_ids))
        else:
            rc = lib.axon_start_nrt_profile(None, 0)
        if rc != 0:
            raise RuntimeError(f"axon_start_nrt_profile rc={rc}")
        try:
            yield
        finally:
            n = lib.axon_stop_nrt_profile(str(output_dir).encode())
            print(f"ntff profile: {n} file(s) written to {output_dir}")

    mod.set_axon_ntff_profile_hook(_hook)


_NC_CACHE = None
_BUILD_KW = {}


def _get_nc():
    global _NC_CACHE
    if _NC_CACHE is None:
        _NC_CACHE = build(**_BUILD_KW)
    return _NC_CACHE


def _make_in_maps(input, weight, lut, scales_and_zeros, bias):
    bf16 = ml_dtypes.bfloat16
    x = np.asarray(input, dtype=np.float32)
    xT = np.ascontiguousarray(x.T).astype(bf16)  # [I, M]
    codes = np.asarray(weight, dtype=np.int32)
    lut = np.asarray(lut, dtype=np.float32)
    sz = np.asarray(scales_and_zeros, dtype=np.float32)
    bias = np.asarray(bias, dtype=np.float32)
    scaleT = np.ascontiguousarray(sz[..., 0].T)  # [O, I//G]
    zeroT = np.ascontiguousarray(sz[..., 1].T)

    # pair interpolation tables: u_j = c*delta_j + gamma_j (j = c>>1)
    base = lut[:, 0::2]                          # [O, 8]
    dtab = lut[:, 1::2] - base                   # delta_j
    gtab = base - dtab * (2.0 * np.arange(8, dtype=np.float32))

    in_maps = []
    for c in range(NCORES):
        osl = slice(c * OSH, (c + 1) * OSH)
        cs = codes[osl]
        predq = np.empty((OSH, 3, I), dtype=np.uint8)
        predq[:, 0, :] = (cs >> 1) & 1                    # bit1
        predq[:, 1, :] = (cs >> 2) & 1                    # bit2
        predq[:, 2, :] = (cs >> 3) & 1                    # bit3
        tblv = np.concatenate([dtab[osl], gtab[osl],
                               scaleT[osl], zeroT[osl]],
                              axis=1).astype(np.float32)
        in_maps.append({
            "xT": xT,
            "cpl": np.ascontiguousarray(cs).astype(bf16),
            "predq": predq,
            "tbl": np.ascontiguousarray(tblv),
            "bias": np.ascontiguousarray(
                bias[osl].reshape(OT, P).T),
        })
    return in_maps


def run(input, weight, lut, scales_and_zeros, bias, trace=False, tmpdir=None):
    if trace:
        _install_ntff_shim()
        import concourse.bass_utils as _bu
        _bu.upload_artifacts = lambda d: d  # zero-egress container
    nc = _get_nc()
    in_maps = _make_in_maps(input, weight, lut, scales_and_zeros, bias)
    res = run_bass_kernel_spmd(
        nc, in_maps, list(range(NCORES)), trace=trace, tmpdir=tmpdir)
    y = np.empty((M, O), dtype=np.float32)
    for c in range(NCORES):
        yt = np.asarray(res.results[c]["yt"])  # [OSH, M] bf16
        y[:, c * OSH:(c + 1) * OSH] = yt.astype(np.float32).T
    return y, res


def kernel(input, weight, lut, scales_and_zeros, bias):
    orig_shape = np.asarray(input).shape
    y, _ = run(input, weight, lut, scales_and_zeros, bias, trace=False)
    return y.reshape(*orig_shape[:-1], O)
